# revision 8
# baseline (speedup 1.0000x reference)
"""Distributed Trainium2 kernel for nn_Attention_29892972380716.

Sharding: batch*head parallel — 8 cores, each owns (b, h0=2*(c%4), h1=h0+1)
with b = c//4.  Each core computes its two heads' q/k/v projections,
flash-style attention in the S^T layout (ktok on partitions, softmax
denominator via a ones-column in the AV matmul), and a partial output
projection.  Host sums the 4 per-batch partials and adds b_proj.

All layouts are chosen so that every on-chip op is partition-aligned:
 - pb0 = (q_h0 rows 0:64 | q_h1 rows 64:128), pb1 = k-pair, pb2 = v-pair.
 - Mirrored copies (qm/km, opposite halves) made by SBUF->SBUF DMA enable
   2x row-tile packing of the S^T matmul (K=64 on rows 0:63 and 64:127).
 - Softmax normalization is deferred: unnormalized out^T feeds the output
   projection, and 1/den is applied per-token (per-partition scalar) on
   the projection PSUM, per head, before summing heads.
"""

import numpy as np
import ml_dtypes

import concourse.mybir as mybir
from concourse import bacc
from concourse.tile import TileContext
from concourse.bass_utils import run_bass_kernel_spmd
from concourse.masks import make_identity

F32 = mybir.dt.float32
BF16 = mybir.dt.bfloat16
BF = ml_dtypes.bfloat16

B, N, C, H = 2, 4096, 512, 8
HD = C // H          # 64
SCALE = HD ** -0.5   # 0.125
NT = N // 128        # 32 token chunks of 128
NJ = N // 512        # 8 qtok chunks of 512
EXP = mybir.ActivationFunctionType.Exp

_CACHE = {}


def build():
    nc = bacc.Bacc("TRN2", target_bir_lowering=False, debug=False)

    xt = nc.declare_dram_parameter("xt", [C, N], BF16, isOutput=False)
    wt = nc.declare_dram_parameter("wt", [C, 6 * HD], BF16, isOutput=False)
    hatT = nc.declare_dram_parameter("hatT", [3, 128, N], F32, isOutput=False)
    wp = nc.declare_dram_parameter("wp", [2, HD, C], BF16, isOutput=False)
    qkvT = nc.declare_dram_parameter("qkvT", [3, 128, N], F32, isOutput=True)
    y = nc.declare_dram_parameter("y", [N, C], F32, isOutput=True)

    with TileContext(nc) as tc:
        with tc.tile_pool(name="const", bufs=1) as constp, \
             tc.tile_pool(name="big", bufs=1) as bigp:
            id_f32 = constp.tile([128, 128], F32)
            make_identity(nc, id_f32)
            id_bf = constp.tile([128, 128], BF16)
            make_identity(nc, id_bf)

            xt_sb = bigp.tile([128, 4, N], BF16, tag="xt_sb")
            for cc in range(4):
                nc.sync.dma_start(out=xt_sb[:, cc, :], in_=xt[cc * 128:(cc + 1) * 128, :])
            wt_sb = bigp.tile([128, 4, 6 * HD], BF16, tag="wt_sb")
            nc.sync.dma_start(out=wt_sb, in_=wt.rearrange("(c p) w -> p c w", p=128))
            wp_sb = bigp.tile([HD, 2, C], BF16, tag="wp_sb")
            nc.sync.dma_start(out=wp_sb, in_=wp.rearrange("h d c -> d h c"))

            # pair tiles: pb[0]=q (h0 rows 0:64, h1 rows 64:128), pb[1]=k, pb[2]=v
            pb = [bigp.tile([128, N], BF16, name=f"pb{i}", tag=f"pb{i}") for i in range(3)]
            qm = bigp.tile([128, N], BF16, tag="qm")  # mirrored q (halves swapped)
            km = bigp.tile([128, N], BF16, tag="km")  # mirrored k
            vaug = bigp.tile([128, 2, NT, HD + 1], BF16, tag="vaug")
            outTU = bigp.tile([HD, 2, N], BF16, tag="outTU")  # unnormalized out^T
            recipc = bigp.tile([128, 2, NT], F32, tag="recipc")  # 1/den columns

            # ---- phase 1: qkv projections + hat add + outputs + casts ----
            # Pair order k, q, v so attention (which needs full k + q
            # mirrors) can start as early as possible.
            with tc.tile_pool(name="hatp", bufs=8) as hatp, \
                 tc.tile_pool(name="stagep", bufs=6) as stagep, \
                 tc.tile_pool(name="psA", bufs=2, space="PSUM") as psA, \
                 tc.tile_pool(name="psT", bufs=4, space="PSUM") as psT:
                nc.vector.memset(vaug[:, :, :, HD:HD + 1], 1.0)
                for p in (1, 0, 2):
                    for j in range(NJ):
                        js = slice(j * 512, (j + 1) * 512)
                        ps = psA.tile([128, 512], F32)
                        for cc in range(4):
                            nc.tensor.matmul(
                                ps,
                                lhsT=wt_sb[:, cc, p * 128:(p + 1) * 128],
                                rhs=xt_sb[:, cc, js],
                                start=(cc == 0), stop=(cc == 3),
                            )
                        hat = hatp.tile([128, 512], F32)
                        nc.sync.dma_start(out=hat, in_=hatT[p, :, js])
                        st = stagep.tile([128, 512], F32)
                        nc.vector.tensor_add(out=st, in0=ps, in1=hat)
                        nc.sync.dma_start(out=qkvT[p, :, js], in_=st)
                        nc.gpsimd.tensor_copy(out=pb[p][:, js], in_=st)
                        # per-chunk mirrored copies (swap halves) SBUF->SBUF
                        if p == 0:
                            nc.sync.dma_start(out=qm[64:128, js], in_=pb[0][0:64, js])
                            nc.sync.dma_start(out=qm[0:64, js], in_=pb[0][64:128, js])
                        elif p == 1:
                            nc.sync.dma_start(out=km[64:128, js], in_=pb[1][0:64, js])
                            nc.sync.dma_start(out=km[0:64, js], in_=pb[1][64:128, js])
                        else:
                            # v transposes into vaug as soon as each chunk lands
                            for hh in range(2):
                                src = pb[2][0:64, :] if hh == 0 else pb[2][64:128, :]
                                idb = id_bf[0:64, 0:64] if hh == 0 else id_bf[64:128, 64:128]
                                for i in range(4 * j, 4 * (j + 1)):
                                    tp = psT.tile([128, HD], BF16)
                                    nc.tensor.transpose(tp, src[:, i * 128:(i + 1) * 128], idb)
                                    nc.vector.tensor_copy(out=vaug[:, hh, i, 0:HD], in_=tp)

            # ---- phase 2+3: attention + fused output projection ----
            with tc.tile_pool(name="psS", bufs=2, space="PSUM") as psS, \
                 tc.tile_pool(name="psAVD", bufs=1, space="PSUM") as psAVD, \
                 tc.tile_pool(name="psY", bufs=1, space="PSUM") as psY, \
                 tc.tile_pool(name="exp", bufs=3) as expp, \
                 tc.tile_pool(name="denp", bufs=2) as denp, \
                 tc.tile_pool(name="ysum", bufs=4) as ysump:
                for j in range(NJ):
                    js = slice(j * 512, (j + 1) * 512)
                    for hh in range(2):
                        if hh == 0:
                            q_lo, k_lo = pb[0][0:64, :], pb[1][0:64, :]       # base 0
                            q_hi, k_hi = qm[64:128, :], km[64:128, :]         # base 64
                        else:
                            q_hi, k_hi = pb[0][64:128, :], pb[1][64:128, :]   # base 64
                            q_lo, k_lo = qm[0:64, :], km[0:64, :]             # base 0
                        av = psAVD.tile([HD + 1, 512], F32, tag="av", bufs=1)
                        for g in range(NT // 2):
                            sps = psS.tile([128, 1024], F32)
                            for u in range(2):
                                kt = 2 * g + u
                                ksl = slice(kt * 128, (kt + 1) * 128)
                                if u == 0:
                                    nc.tensor.matmul(
                                        sps[:, 0:512], lhsT=k_lo[:, ksl], rhs=q_lo[:, js],
                                        start=True, stop=True, tile_position=(0, 0),
                                    )
                                else:
                                    nc.tensor.matmul(
                                        sps[:, 512:1024], lhsT=k_hi[:, ksl], rhs=q_hi[:, js],
                                        start=True, stop=True, tile_position=(64, 0),
                                    )
                            ex = expp.tile([128, 1024], BF16)
                            nc.scalar.activation(out=ex, in_=sps, func=EXP, scale=SCALE)
                            for u in range(2):
                                kt = 2 * g + u
                                nc.tensor.matmul(
                                    av,
                                    lhsT=vaug[:, hh, kt, :],
                                    rhs=ex[:, u * 512:(u + 1) * 512],
                                    start=(kt == 0), stop=(kt == NT - 1),
                                )
                        # unnormalized out^T slice
                        nc.vector.tensor_copy(out=outTU[:, hh, js], in_=av[0:HD, :])
                        # denominator row -> columns -> reciprocal
                        den = denp.tile([HD + 1, 512], F32)
                        nc.vector.tensor_copy(out=den[HD:HD + 1, :], in_=av[HD:HD + 1, :])
                        dps = psAVD.tile([128, 4], F32, tag="dps", bufs=1)
                        for t in range(4):
                            nc.tensor.transpose(
                                dps[:, t:t + 1],
                                den[HD:HD + 1, t * 128:(t + 1) * 128],
                                id_f32[64:65, 64:65],
                            )
                        nc.vector.reciprocal(out=recipc[:, hh, j * 4:(j + 1) * 4], in_=dps)
                    # output projection for this j's 4 token chunks
                    for nt in range(4 * j, 4 * (j + 1)):
                        nsl = slice(nt * 128, (nt + 1) * 128)
                        ps0 = psY.tile([128, C], F32, tag="ps0", bufs=1)
                        nc.tensor.matmul(ps0, lhsT=outTU[:, 0, nsl], rhs=wp_sb[:, 0, :],
                                         start=True, stop=True)
                        ps1 = psY.tile([128, C], F32, tag="ps1", bufs=1)
                        nc.tensor.matmul(ps1, lhsT=outTU[:, 1, nsl], rhs=wp_sb[:, 1, :],
                                         start=True, stop=True)
                        y0 = ysump.tile([128, C], F32, tag="y0")
                        nc.vector.tensor_scalar_mul(out=y0, in0=ps0, scalar1=recipc[:, 0, nt:nt + 1])
                        y1 = ysump.tile([128, C], F32, tag="y1")
                        nc.vector.tensor_scalar_mul(out=y1, in0=ps1, scalar1=recipc[:, 1, nt:nt + 1])
                        ys = ysump.tile([128, C], F32, tag="ys")
                        nc.vector.tensor_add(out=ys, in0=y0, in1=y1)
                        nc.sync.dma_start(out=y[nsl, :], in_=ys)

    nc.compile()
    return nc


def _get_nc():
    if "nc" not in _CACHE:
        _CACHE["nc"] = build()
    return _CACHE["nc"]


def _make_in_maps(x, q_hat, k_hat, v_hat, W_qkv, W_proj):
    WqT = [np.ascontiguousarray(W_qkv[h * HD:(h + 1) * HD, :].T) for h in range(H)]
    WkT = [np.ascontiguousarray(W_qkv[C + h * HD:C + (h + 1) * HD, :].T) for h in range(H)]
    WvT = [np.ascontiguousarray(W_qkv[2 * C + h * HD:2 * C + (h + 1) * HD, :].T) for h in range(H)]
    WpT = np.ascontiguousarray(W_proj.T)  # [cin, cout]

    in_maps = []
    for c in range(8):
        b, h0 = c // 4, 2 * (c % 4)
        h1 = h0 + 1
        xt = np.ascontiguousarray(x[b].T).astype(BF)
        wt = np.concatenate(
            [WqT[h0], WqT[h1], WkT[h0], WkT[h1], WvT[h0], WvT[h1]], axis=1
        ).astype(BF)
        hatT = np.stack([
            np.concatenate([q_hat[b, h0].T, q_hat[b, h1].T], axis=0),
            np.concatenate([k_hat[b, h0].T, k_hat[b, h1].T], axis=0),
            np.concatenate([v_hat[b, h0].T, v_hat[b, h1].T], axis=0),
        ]).astype(np.float32)
        wp = np.stack([WpT[h0 * HD:(h0 + 1) * HD, :], WpT[h1 * HD:(h1 + 1) * HD, :]]).astype(BF)
        in_maps.append({"xt": np.ascontiguousarray(xt),
                        "wt": np.ascontiguousarray(wt),
                        "hatT": np.ascontiguousarray(hatT),
                        "wp": np.ascontiguousarray(wp)})
    return in_maps


def _assemble(results, b_proj):
    q = np.empty((B, H, N, HD), np.float32)
    k = np.empty((B, H, N, HD), np.float32)
    v = np.empty((B, H, N, HD), np.float32)
    out = np.zeros((B, N, C), np.float32)
    for c, r in enumerate(results):
        b, h0 = c // 4, 2 * (c % 4)
        h1 = h0 + 1
        t = np.asarray(r["qkvT"], np.float32)
        q[b, h0], q[b, h1] = t[0, 0:HD].T, t[0, HD:128].T
        k[b, h0], k[b, h1] = t[1, 0:HD].T, t[1, HD:128].T
        v[b, h0], v[b, h1] = t[2, 0:HD].T, t[2, HD:128].T
        out[b] += np.asarray(r["y"], np.float32)
    out += b_proj[None, None, :]
    return (out, q, k, v)


def kernel(x, q_hat, k_hat, v_hat, W_qkv, W_proj, b_proj):
    x = np.asarray(x, np.float32)
    q_hat = np.asarray(q_hat, np.float32)
    k_hat = np.asarray(k_hat, np.float32)
    v_hat = np.asarray(v_hat, np.float32)
    W_qkv = np.asarray(W_qkv, np.float32)
    W_proj = np.asarray(W_proj, np.float32)
    b_proj = np.asarray(b_proj, np.float32)

    nc = _get_nc()
    in_maps = _make_in_maps(x, q_hat, k_hat, v_hat, W_qkv, W_proj)
    res = run_bass_kernel_spmd(nc, in_maps, core_ids=list(range(8)))
    return _assemble(res.results, b_proj)
